# revision 1
# baseline (speedup 1.0000x reference)
"""GQA attention kernel for Trainium2, 8 NeuronCores.

Sharding: 2 batches x 4 kv-head groups = 8 cores. Each core computes, for its
batch b and kv group g (4 query heads, 1 kv head):
    Q = x_b @ Wq[:, g]     (512 cols)      K = x_b @ Wk[:, g] (128 cols)
    V = x_b @ Wv[:, g]     (128 cols)
    A_h = softmax_causal(Q_h K^T / sqrt(128)) V        (h = 4 heads)
    Y_partial = concat_h(A_h) @ Wo[rows g]             [2048, 2048]
Host sums the 4 partials per batch and adds bo.

Device layout choices (all matmul operands natural, no transposes in hot loop):
  xT [d, t] fed from host; QT/KT computed transposed ([e, t]); V non-transposed
  via PE transpose of VT; scores computed transposed ST [s, t] so that
  AV (lhsT=V[s,e], rhs=expST[s,t]) and O-proj (lhsT=attnT[c,t], rhs=Wo[c,f])
  need no on-device transposition. Softmax denominators via ones-vector
  matmuls; normalization deferred to attnT evacuation using a PE-broadcast
  of 1/Z. Causal masking: only lower-triangular 128x512 score blocks are
  computed; diagonal blocks masked multiplicatively post-exp.
Compute dtype bf16 (inputs cast on host), accumulation f32.
"""

import os
import sys

sys.path.insert(0, "/opt/trn_rl_repo")

import numpy as np
import ml_dtypes

import concourse.bass as bass
from concourse import bacc
import concourse.tile as tile
from concourse import mybir
from concourse.bass_utils import run_bass_kernel_spmd

BF = mybir.dt.bfloat16
F32 = mybir.dt.float32

D = 2048        # d_model
T = 2048        # seq len
B = 2
NUM_HEADS = 16
NUM_KV = 4
DH = 128        # head dim
HPG = NUM_HEADS // NUM_KV   # 4 query heads per core
EG = HPG * DH               # 512 q-channels per core
TS = 512                    # t-slice width (phase A psum tiles, phase B rhs)
NT = T // TS                # 4
NJ = D // 128               # 16 contraction chunks / s-tiles
SCALE = 1.0 / float(np.sqrt(DH))

_NC_CACHE = {}


def build_nc():
    if "nc" in _NC_CACHE:
        return _NC_CACHE["nc"]
    nc = bass.Bass()
    xT = nc.dram_tensor("xT", [D, T], BF, kind="ExternalInput").ap()
    Wq = nc.dram_tensor("Wq", [D, EG], BF, kind="ExternalInput").ap()
    Wk = nc.dram_tensor("Wk", [D, DH], BF, kind="ExternalInput").ap()
    Wv = nc.dram_tensor("Wv", [D, DH], BF, kind="ExternalInput").ap()
    Wo = nc.dram_tensor("Wo", [EG, D], BF, kind="ExternalInput").ap()
    cpack_d = nc.dram_tensor("cpack", [128, 2305], BF, kind="ExternalInput").ap()
    bpack_d = nc.dram_tensor("bpack", [128, 6], F32, kind="ExternalInput").ap()
    y = nc.dram_tensor("y", [T, D], F32, kind="ExternalOutput").ap()

    with tile.TileContext(nc) as tc:
        with (
            tc.tile_pool(name="consts", bufs=1) as consts,
            tc.tile_pool(name="persist", bufs=1) as persist,
            tc.tile_pool(name="wpool", bufs=1) as wpool,
            tc.tile_pool(name="xpool", bufs=64) as xpool,
            tc.tile_pool(name="expp", bufs=3) as expp,
            tc.tile_pool(name="attp", bufs=8) as attp,
            tc.tile_pool(name="ypool", bufs=4) as ypool,
            tc.tile_pool(name="small", bufs=8) as small,
        ):
            # ---- constants: two packed host tensors, one DMA each ----------
            # cpack: identity(128) | maskA(1024) | maskB(1024) | ones(1) | ones(128)
            cpack = consts.tile([128, 2305], BF)
            nc.sync.dma_start(out=cpack, in_=cpack_d)
            bpack = consts.tile([128, 6], F32)
            nc.sync.dma_start(out=bpack, in_=bpack_d)
            identity = cpack[:, 0:128]
            maskA = cpack[:, 128:1152]     # diag-block masks, offsets 0,1
            maskB = cpack[:, 1152:2176]    # offsets 2,3
            ones_s = cpack[:, 2176:2177]   # lhsT for column sums
            ones_r = cpack[0:1, 2177:2305]  # lhsT for partition bcast
            bq_sb = bpack[:, 0:HPG]
            bk_sb = bpack[:, HPG:HPG + 1]
            bv_sb = bpack[:, HPG + 1:HPG + 2]
            # Pre-touch on DVE: later DVE consumers then carry only one wait
            # (walrus allows a single sync wait on DVE tensor_scalar ops).
            pt = consts.tile([128, 16], BF)
            nc.vector.tensor_copy(out=pt, in_=cpack[:, 0:16])
            ptf = consts.tile([128, 6], F32)
            nc.vector.tensor_copy(out=ptf, in_=bpack)

            # ---- persistent activations -----------------------------------
            QT = [persist.tile([128, T], BF, tag=f"QT{h}", name=f"QT{h}") for h in range(HPG)]
            KT = persist.tile([128, T], BF, tag="KT")
            V = persist.tile([128, NJ, DH], BF, tag="V")       # [s%128, j, e]
            Wq_sb = wpool.tile([128, NJ, EG], BF, tag="Wq")
            Wk_sb = wpool.tile([128, NJ, DH], BF, tag="Wk")
            Wv_sb = wpool.tile([128, NJ, DH], BF, tag="Wv")
            Wo_sb = wpool.tile([128, HPG, D], BF, tag="Wo")    # [c%128, h, f]
            for h in range(HPG):
                nc.sync.dma_start(out=Wo_sb[:, h, :], in_=Wo[h * 128:(h + 1) * 128, :])

            # ---- phase A: projections QT/KT/V ------------------------------
            with (
                tc.tile_pool(name="psA", bufs=1, space="PSUM") as psA,
                tc.tile_pool(name="psAv", bufs=2, space="PSUM") as psAv,
            ):
                warm = psAv.tile([128, 128], BF, tag="v_ps")
                nc.tensor.transpose(warm, identity, identity)
                for Tt in range(NT):
                    tsl = slice(Tt * TS, (Tt + 1) * TS)
                    xa = []
                    for j in range(NJ):
                        xt = xpool.tile([128, TS], BF, tag="xa")
                        nc.sync.dma_start(out=xt, in_=xT[j * 128:(j + 1) * 128, tsl])
                        xa.append(xt)
                        if Tt == 0:
                            nc.sync.dma_start(out=Wq_sb[:, j, :], in_=Wq[j * 128:(j + 1) * 128, :])
                            nc.sync.dma_start(out=Wk_sb[:, j, :], in_=Wk[j * 128:(j + 1) * 128, :])
                            nc.sync.dma_start(out=Wv_sb[:, j, :], in_=Wv[j * 128:(j + 1) * 128, :])
                    # one output tile at a time so evacuation overlaps compute
                    for h in range(HPG):
                        qt_ps = psA.tile([128, TS], F32, tag=f"qt{h}")
                        for j in range(NJ):
                            nc.tensor.matmul(
                                qt_ps, Wq_sb[:, j, h * 128:(h + 1) * 128], xa[j],
                                start=(j == 0), stop=(j == NJ - 1),
                            )
                        nc.vector.tensor_scalar_add(
                            out=QT[h][:, tsl], in0=qt_ps,
                            scalar1=bq_sb[:, h:h + 1],
                        )
                    kt_ps = psA.tile([128, TS], F32, tag="kt")
                    for j in range(NJ):
                        nc.tensor.matmul(kt_ps, Wk_sb[:, j, :], xa[j],
                                         start=(j == 0), stop=(j == NJ - 1))
                    nc.vector.tensor_scalar_add(
                        out=KT[:, tsl], in0=kt_ps, scalar1=bk_sb,
                    )
                    vt_ps = psA.tile([128, TS], F32, tag="vt")
                    for j in range(NJ):
                        nc.tensor.matmul(vt_ps, Wv_sb[:, j, :], xa[j],
                                         start=(j == 0), stop=(j == NJ - 1))
                    vt_sb = small.tile([128, TS], BF, tag="vt_sb")
                    nc.vector.tensor_scalar_add(
                        out=vt_sb, in0=vt_ps, scalar1=bv_sb,
                    )
                    # VT [e, t] -> V [t, e] per 128-block via PE transpose
                    for k in range(TS // 128):
                        v_ps = psAv.tile([128, 128], BF, tag="v_ps")
                        nc.tensor.transpose(v_ps, vt_sb[:, k * 128:(k + 1) * 128], identity)
                        nc.vector.tensor_copy(out=V[:, Tt * 4 + k, :], in_=v_ps)

            # ---- phase B/C: attention + output projection ------------------
            with (
                tc.tile_pool(name="psst", bufs=2, space="PSUM") as psst,
                tc.tile_pool(name="psat", bufs=1, space="PSUM") as psat,
                tc.tile_pool(name="psz", bufs=1, space="PSUM") as psz,
                tc.tile_pool(name="psy", bufs=2, space="PSUM") as psy,
            ):
                for Tt in range(NT):
                    tsl = slice(Tt * TS, (Tt + 1) * TS)
                    att_sb = []
                    for h in range(HPG):
                        njj = 4 * Tt + 4          # s-tiles 0 .. 4*Tt+3
                        ngr = njj // 2
                        at_ps = psat.tile([128, TS], F32, tag="at")
                        z_ps = psz.tile([1, TS], F32, tag="z")
                        for g in range(ngr):
                            j0 = 2 * g
                            st = psst.tile([128, 1024], F32, tag="st")
                            for half in range(2):
                                j = j0 + half
                                nc.tensor.matmul(
                                    st[:, half * 512:(half + 1) * 512],
                                    KT[:, j * 128:(j + 1) * 128],
                                    QT[h][:, tsl],
                                    start=True, stop=True,
                                )
                            ex = expp.tile([128, 1024], BF, tag="ex")
                            nc.scalar.activation(
                                out=ex, in_=st,
                                func=mybir.ActivationFunctionType.Exp,
                                scale=SCALE,
                            )
                            if g == ngr - 2:
                                nc.vector.tensor_mul(ex, ex, maskA)
                            elif g == ngr - 1:
                                nc.vector.tensor_mul(ex, ex, maskB)
                            for half in range(2):
                                j = j0 + half
                                exh = ex[:, half * 512:(half + 1) * 512]
                                nc.tensor.matmul(
                                    z_ps, ones_s, exh,
                                    start=(j == 0), stop=(j == njj - 1),
                                )
                                nc.tensor.matmul(
                                    at_ps, V[:, j, :], exh,
                                    start=(j == 0), stop=(j == njj - 1),
                                )
                        zr = small.tile([1, TS], F32, tag="zr")
                        nc.vector.reciprocal(out=zr, in_=z_ps)
                        zrb = small.tile([1, TS], BF, tag="zrb")
                        nc.vector.tensor_copy(out=zrb, in_=zr)
                        zb_ps = psz.tile([128, TS], F32, tag="z")
                        nc.tensor.matmul(zb_ps, ones_r, zrb,
                                         start=True, stop=True)
                        zb_sb = small.tile([128, TS], BF, tag="zb_sb")
                        nc.vector.tensor_copy(out=zb_sb, in_=zb_ps)
                        at_sb = attp.tile([128, TS], BF, tag="at_sb")
                        nc.vector.tensor_mul(at_sb, at_ps, zb_sb)
                        att_sb.append(at_sb)
                    # output projection for these 512 rows
                    for fs in range(4):
                        fsl = slice(fs * 512, (fs + 1) * 512)
                        for tt in range(4):
                            y_ps = psy.tile([128, 512], F32, tag="y")
                            for h in range(HPG):
                                nc.tensor.matmul(
                                    y_ps,
                                    att_sb[h][:, tt * 128:(tt + 1) * 128],
                                    Wo_sb[:, h, fsl],
                                    start=(h == 0), stop=(h == HPG - 1),
                                )
                            y_sb = ypool.tile([128, 512], F32, tag="y_sb")
                            nc.vector.tensor_copy(out=y_sb, in_=y_ps)
                            nc.sync.dma_start(
                                out=y[Tt * TS + tt * 128: Tt * TS + (tt + 1) * 128, fsl],
                                in_=y_sb,
                            )
    from concourse.bacc import _bass_rust
    _bass_rust.move_matmul_waits_to_ldweights(nc.m)
    _bass_rust.generate_event_semaphores(nc)
    _NC_CACHE["nc"] = nc
    return nc


def _cpack():
    bf = ml_dtypes.bfloat16
    ident = np.eye(128, dtype=np.float32)
    tc = np.arange(512)[None, :]
    s = np.arange(128)[:, None]
    def mk(o0, o1):
        return np.concatenate(
            [(tc >= o0 * 128 + s), (tc >= o1 * 128 + s)], axis=1
        ).astype(np.float32)
    ones = np.ones((128, 129), np.float32)
    return np.concatenate([ident, mk(0, 1), mk(2, 3), ones], axis=1).astype(bf)


def make_in_maps(x, Wq, bq, Wk, bk, Wv, bv, Wo, bo):
    bf = ml_dtypes.bfloat16
    cpack = _cpack()
    xTb = [np.ascontiguousarray(np.asarray(x[b]).T).astype(bf) for b in range(B)]
    Wq = np.asarray(Wq); Wk = np.asarray(Wk); Wv = np.asarray(Wv); Wo = np.asarray(Wo)
    bq = np.asarray(bq, dtype=np.float32); bk = np.asarray(bk, dtype=np.float32)
    bv = np.asarray(bv, dtype=np.float32)
    in_maps = []
    for c in range(8):
        b, g = divmod(c, NUM_KV)
        in_maps.append({
            "xT": xTb[b],
            "Wq": np.ascontiguousarray(Wq[:, g * EG:(g + 1) * EG]).astype(bf),
            "Wk": np.ascontiguousarray(Wk[:, g * DH:(g + 1) * DH]).astype(bf),
            "Wv": np.ascontiguousarray(Wv[:, g * DH:(g + 1) * DH]).astype(bf),
            "Wo": np.ascontiguousarray(Wo[g * EG:(g + 1) * EG, :]).astype(bf),
            "bq": np.ascontiguousarray(bq[g * EG:(g + 1) * EG]).reshape(EG, 1),
            "bk": np.ascontiguousarray(bk[g * DH:(g + 1) * DH]).reshape(DH, 1),
            "cpack": cpack,
            "bpack": np.concatenate(
                [bq[g * EG:(g + 1) * EG].reshape(4, DH).T,
                 bk[g * DH:(g + 1) * DH].reshape(DH, 1),
                 bv[g * DH:(g + 1) * DH].reshape(DH, 1)], axis=1
            ).astype(np.float32),
        })
    return in_maps


def gather(results, bo):
    bo = np.asarray(bo, dtype=np.float32)
    out = np.empty((B, T, D), dtype=np.float32)
    for b in range(B):
        acc = results[b * NUM_KV]["y"].astype(np.float32)
        for g in range(1, NUM_KV):
            acc = acc + results[b * NUM_KV + g]["y"].astype(np.float32)
        out[b] = acc + bo[None, :]
    return out


def kernel(x, Wq, bq, Wk, bk, Wv, bv, Wo, bo):
    nc = build_nc()
    in_maps = make_in_maps(x, Wq, bq, Wk, bk, Wv, bv, Wo, bo)
    last = None
    for attempt in range(3):
        try:
            res = run_bass_kernel_spmd(nc, in_maps, list(range(8)))
            return gather(res.results, bo)
        except Exception as e:  # transient NRT_EXEC_UNIT_UNRECOVERABLE
            last = e
            import time as _t
            _t.sleep(10)
    raise last



# revision 2
# speedup vs baseline: 14233.2076x; 14233.2076x over previous
"""GQA attention kernel for Trainium2, 8 NeuronCores.

Sharding: 2 batches x 4 kv-head groups = 8 cores. Each core computes, for its
batch b and kv group g (4 query heads, 1 kv head):
    Q = x_b @ Wq[:, g]     (512 cols)      K = x_b @ Wk[:, g] (128 cols)
    V = x_b @ Wv[:, g]     (128 cols)
    A_h = softmax_causal(Q_h K^T / sqrt(128)) V        (h = 4 heads)
    Y_partial = concat_h(A_h) @ Wo[rows g]             [2048, 2048]
Host sums the 4 partials per batch and adds bo.

Device layout choices (all matmul operands natural, no transposes in hot loop):
  xT [d, t] fed from host; QT/KT computed transposed ([e, t]); V non-transposed
  via PE transpose of VT; scores computed transposed ST [s, t] so that
  AV (lhsT=V[s,e], rhs=expST[s,t]) and O-proj (lhsT=attnT[c,t], rhs=Wo[c,f])
  need no on-device transposition. Softmax denominators via ones-vector
  matmuls; normalization deferred to attnT evacuation using a PE-broadcast
  of 1/Z. Causal masking: only lower-triangular 128x512 score blocks are
  computed; diagonal blocks masked multiplicatively post-exp.
Compute dtype bf16 (inputs cast on host), accumulation f32.
"""

import os
import sys

sys.path.insert(0, "/opt/trn_rl_repo")

import numpy as np
import ml_dtypes

import concourse.bass as bass
from concourse import bacc
import concourse.tile as tile
from concourse import mybir
from concourse.bass_utils import run_bass_kernel_spmd

BF = mybir.dt.bfloat16
F32 = mybir.dt.float32

D = 2048        # d_model
T = 2048        # seq len
B = 2
NUM_HEADS = 16
NUM_KV = 4
DH = 128        # head dim
HPG = NUM_HEADS // NUM_KV   # 4 query heads per core
EG = HPG * DH               # 512 q-channels per core
TS = 512                    # t-slice width (phase A psum tiles, phase B rhs)
NT = T // TS                # 4
NJ = D // 128               # 16 contraction chunks / s-tiles
SCALE = 1.0 / float(np.sqrt(DH))

_NC_CACHE = {}


def _emit_iter(nc, tc, xT, Wq, Wk, Wv, Wo, y, ch):
    """One full kernel iteration (weight loads + phases A..C)."""
    identity = ch["identity"]; maskA = ch["maskA"]; maskB = ch["maskB"]
    ones_s = ch["ones_s"]; ones_r = ch["ones_r"]
    bq_sb = ch["bq_sb"]; bk_sb = ch["bk_sb"]; bv_sb = ch["bv_sb"]
    with (
        tc.tile_pool(name="persist", bufs=1) as persist,
        tc.tile_pool(name="wpool", bufs=1) as wpool,
        tc.tile_pool(name="xpool", bufs=64) as xpool,
        tc.tile_pool(name="expp", bufs=3) as expp,
        tc.tile_pool(name="attp", bufs=8) as attp,
        tc.tile_pool(name="ypool", bufs=4) as ypool,
        tc.tile_pool(name="small", bufs=8) as small,
    ):
        # ---- persistent activations -----------------------------------
        QT = [persist.tile([128, T], BF, tag=f"QT{h}", name=f"QT{h}") for h in range(HPG)]
        KT = persist.tile([128, T], BF, tag="KT")
        V = persist.tile([128, NJ, DH], BF, tag="V")       # [s%128, j, e]
        Wq_sb = wpool.tile([128, NJ, EG], BF, tag="Wq")
        Wk_sb = wpool.tile([128, NJ, DH], BF, tag="Wk")
        Wv_sb = wpool.tile([128, NJ, DH], BF, tag="Wv")
        Wo_sb = wpool.tile([128, HPG, D], BF, tag="Wo")    # [c%128, h, f]
        for h in range(HPG):
            nc.sync.dma_start(out=Wo_sb[:, h, :], in_=Wo[h * 128:(h + 1) * 128, :])

        # ---- phase A: projections QT/KT/V ------------------------------
        with (
            tc.tile_pool(name="psA", bufs=1, space="PSUM") as psA,
            tc.tile_pool(name="psAv", bufs=2, space="PSUM") as psAv,
        ):
            warm = psAv.tile([128, 128], BF, tag="v_ps")
            nc.tensor.transpose(warm, identity, identity)
            for Tt in range(NT):
                tsl = slice(Tt * TS, (Tt + 1) * TS)
                xa = []
                for j in range(NJ):
                    xt = xpool.tile([128, TS], BF, tag="xa")
                    nc.sync.dma_start(out=xt, in_=xT[j * 128:(j + 1) * 128, tsl])
                    xa.append(xt)
                    if Tt == 0:
                        nc.sync.dma_start(out=Wq_sb[:, j, :], in_=Wq[j * 128:(j + 1) * 128, :])
                        nc.sync.dma_start(out=Wk_sb[:, j, :], in_=Wk[j * 128:(j + 1) * 128, :])
                        nc.sync.dma_start(out=Wv_sb[:, j, :], in_=Wv[j * 128:(j + 1) * 128, :])
                # one output tile at a time so evacuation overlaps compute
                for h in range(HPG):
                    qt_ps = psA.tile([128, TS], F32, tag=f"qt{h}")
                    for j in range(NJ):
                        nc.tensor.matmul(
                            qt_ps, Wq_sb[:, j, h * 128:(h + 1) * 128], xa[j],
                            start=(j == 0), stop=(j == NJ - 1),
                        )
                    nc.vector.tensor_scalar_add(
                        out=QT[h][:, tsl], in0=qt_ps,
                        scalar1=bq_sb[:, h:h + 1],
                    )
                kt_ps = psA.tile([128, TS], F32, tag="kt")
                for j in range(NJ):
                    nc.tensor.matmul(kt_ps, Wk_sb[:, j, :], xa[j],
                                     start=(j == 0), stop=(j == NJ - 1))
                nc.vector.tensor_scalar_add(
                    out=KT[:, tsl], in0=kt_ps, scalar1=bk_sb,
                )
                vt_ps = psA.tile([128, TS], F32, tag="vt")
                for j in range(NJ):
                    nc.tensor.matmul(vt_ps, Wv_sb[:, j, :], xa[j],
                                     start=(j == 0), stop=(j == NJ - 1))
                vt_sb = small.tile([128, TS], BF, tag="vt_sb")
                nc.vector.tensor_scalar_add(
                    out=vt_sb, in0=vt_ps, scalar1=bv_sb,
                )
                # VT [e, t] -> V [t, e] per 128-block via PE transpose
                for k in range(TS // 128):
                    v_ps = psAv.tile([128, 128], BF, tag="v_ps")
                    nc.tensor.transpose(v_ps, vt_sb[:, k * 128:(k + 1) * 128], identity)
                    nc.vector.tensor_copy(out=V[:, Tt * 4 + k, :], in_=v_ps)

        # ---- phase B/C: attention + output projection ------------------
        with (
            tc.tile_pool(name="psst", bufs=2, space="PSUM") as psst,
            tc.tile_pool(name="psat", bufs=1, space="PSUM") as psat,
            tc.tile_pool(name="psz", bufs=1, space="PSUM") as psz,
            tc.tile_pool(name="psy", bufs=2, space="PSUM") as psy,
        ):
            for Tt in range(NT):
                tsl = slice(Tt * TS, (Tt + 1) * TS)
                att_sb = []
                for h in range(HPG):
                    njj = 4 * Tt + 4          # s-tiles 0 .. 4*Tt+3
                    ngr = njj // 2
                    at_ps = psat.tile([128, TS], F32, tag="at")
                    z_ps = psz.tile([1, TS], F32, tag="z")
                    for g in range(ngr):
                        j0 = 2 * g
                        st = psst.tile([128, 1024], F32, tag="st")
                        for half in range(2):
                            j = j0 + half
                            nc.tensor.matmul(
                                st[:, half * 512:(half + 1) * 512],
                                KT[:, j * 128:(j + 1) * 128],
                                QT[h][:, tsl],
                                start=True, stop=True,
                            )
                        ex = expp.tile([128, 1024], BF, tag="ex")
                        nc.scalar.activation(
                            out=ex, in_=st,
                            func=mybir.ActivationFunctionType.Exp,
                            scale=SCALE,
                        )
                        if g == ngr - 2:
                            nc.vector.tensor_mul(ex, ex, maskA)
                        elif g == ngr - 1:
                            nc.vector.tensor_mul(ex, ex, maskB)
                        for half in range(2):
                            j = j0 + half
                            exh = ex[:, half * 512:(half + 1) * 512]
                            nc.tensor.matmul(
                                z_ps, ones_s, exh,
                                start=(j == 0), stop=(j == njj - 1),
                            )
                            nc.tensor.matmul(
                                at_ps, V[:, j, :], exh,
                                start=(j == 0), stop=(j == njj - 1),
                            )
                    zr = small.tile([1, TS], F32, tag="zr")
                    nc.vector.reciprocal(out=zr, in_=z_ps)
                    zrb = small.tile([1, TS], BF, tag="zrb")
                    nc.vector.tensor_copy(out=zrb, in_=zr)
                    zb_ps = psz.tile([128, TS], F32, tag="z")
                    nc.tensor.matmul(zb_ps, ones_r, zrb,
                                     start=True, stop=True)
                    zb_sb = small.tile([128, TS], BF, tag="zb_sb")
                    nc.vector.tensor_copy(out=zb_sb, in_=zb_ps)
                    at_sb = attp.tile([128, TS], BF, tag="at_sb")
                    nc.vector.tensor_mul(at_sb, at_ps, zb_sb)
                    att_sb.append(at_sb)
                # output projection for these 512 rows
                for fs in range(4):
                    fsl = slice(fs * 512, (fs + 1) * 512)
                    for tt in range(4):
                        y_ps = psy.tile([128, 512], F32, tag="y")
                        for h in range(HPG):
                            nc.tensor.matmul(
                                y_ps,
                                att_sb[h][:, tt * 128:(tt + 1) * 128],
                                Wo_sb[:, h, fsl],
                                start=(h == 0), stop=(h == HPG - 1),
                            )
                        y_sb = ypool.tile([128, 512], F32, tag="y_sb")
                        nc.vector.tensor_copy(out=y_sb, in_=y_ps)
                        nc.sync.dma_start(
                            out=y[Tt * TS + tt * 128: Tt * TS + (tt + 1) * 128, fsl],
                            in_=y_sb,
                        )


def build_nc(n_iters=1):
    key = ("nc", n_iters)
    if key in _NC_CACHE:
        return _NC_CACHE[key]
    nc = bass.Bass()
    xT = nc.dram_tensor("xT", [D, T], BF, kind="ExternalInput").ap()
    Wq = nc.dram_tensor("Wq", [D, EG], BF, kind="ExternalInput").ap()
    Wk = nc.dram_tensor("Wk", [D, DH], BF, kind="ExternalInput").ap()
    Wv = nc.dram_tensor("Wv", [D, DH], BF, kind="ExternalInput").ap()
    Wo = nc.dram_tensor("Wo", [EG, D], BF, kind="ExternalInput").ap()
    cpack_d = nc.dram_tensor("cpack", [128, 2305], BF, kind="ExternalInput").ap()
    bpack_d = nc.dram_tensor("bpack", [128, 6], F32, kind="ExternalInput").ap()
    y = nc.dram_tensor("y", [T, D], F32, kind="ExternalOutput").ap()

    with tile.TileContext(nc) as tc:
        with tc.tile_pool(name="consts", bufs=1) as consts:
            # ---- constants: two packed host tensors, one DMA each ----------
            # cpack: identity(128) | maskA(1024) | maskB(1024) | ones(1) | ones(128)
            cpack = consts.tile([128, 2305], BF)
            nc.sync.dma_start(out=cpack, in_=cpack_d)
            bpack = consts.tile([128, 6], F32)
            nc.sync.dma_start(out=bpack, in_=bpack_d)
            ch = {
                "identity": cpack[:, 0:128],
                "maskA": cpack[:, 128:1152],     # diag-block masks, offsets 0,1
                "maskB": cpack[:, 1152:2176],    # offsets 2,3
                "ones_s": cpack[:, 2176:2177],   # lhsT for column sums
                "ones_r": cpack[0:1, 2177:2305],  # lhsT for partition bcast
                "bq_sb": bpack[:, 0:HPG],
                "bk_sb": bpack[:, HPG:HPG + 1],
                "bv_sb": bpack[:, HPG + 1:HPG + 2],
            }
            # Pre-touch on DVE: later DVE consumers then carry only one wait
            # (walrus allows a single sync wait on DVE tensor_scalar ops).
            pt = consts.tile([128, 16], BF)
            nc.vector.tensor_copy(out=pt, in_=cpack[:, 0:16])
            ptf = consts.tile([128, 6], F32)
            nc.vector.tensor_copy(out=ptf, in_=bpack)

            for _ in range(n_iters):
                _emit_iter(nc, tc, xT, Wq, Wk, Wv, Wo, y, ch)
    from concourse.bacc import _bass_rust
    _bass_rust.move_matmul_waits_to_ldweights(nc.m)
    _bass_rust.generate_event_semaphores(nc)
    _NC_CACHE[key] = nc
    return nc


def _cpack():
    bf = ml_dtypes.bfloat16
    ident = np.eye(128, dtype=np.float32)
    tc = np.arange(512)[None, :]
    s = np.arange(128)[:, None]
    def mk(o0, o1):
        return np.concatenate(
            [(tc >= o0 * 128 + s), (tc >= o1 * 128 + s)], axis=1
        ).astype(np.float32)
    ones = np.ones((128, 129), np.float32)
    return np.concatenate([ident, mk(0, 1), mk(2, 3), ones], axis=1).astype(bf)


def make_in_maps(x, Wq, bq, Wk, bk, Wv, bv, Wo, bo):
    bf = ml_dtypes.bfloat16
    cpack = _cpack()
    xTb = [np.ascontiguousarray(np.asarray(x[b]).T).astype(bf) for b in range(B)]
    Wq = np.asarray(Wq); Wk = np.asarray(Wk); Wv = np.asarray(Wv); Wo = np.asarray(Wo)
    bq = np.asarray(bq, dtype=np.float32); bk = np.asarray(bk, dtype=np.float32)
    bv = np.asarray(bv, dtype=np.float32)
    in_maps = []
    for c in range(8):
        b, g = divmod(c, NUM_KV)
        in_maps.append({
            "xT": xTb[b],
            "Wq": np.ascontiguousarray(Wq[:, g * EG:(g + 1) * EG]).astype(bf),
            "Wk": np.ascontiguousarray(Wk[:, g * DH:(g + 1) * DH]).astype(bf),
            "Wv": np.ascontiguousarray(Wv[:, g * DH:(g + 1) * DH]).astype(bf),
            "Wo": np.ascontiguousarray(Wo[g * EG:(g + 1) * EG, :]).astype(bf),
            "bq": np.ascontiguousarray(bq[g * EG:(g + 1) * EG]).reshape(EG, 1),
            "bk": np.ascontiguousarray(bk[g * DH:(g + 1) * DH]).reshape(DH, 1),
            "cpack": cpack,
            "bpack": np.concatenate(
                [bq[g * EG:(g + 1) * EG].reshape(4, DH).T,
                 bk[g * DH:(g + 1) * DH].reshape(DH, 1),
                 bv[g * DH:(g + 1) * DH].reshape(DH, 1)], axis=1
            ).astype(np.float32),
        })
    return in_maps


def gather(results, bo):
    bo = np.asarray(bo, dtype=np.float32)
    out = np.empty((B, T, D), dtype=np.float32)
    for b in range(B):
        acc = results[b * NUM_KV]["y"].astype(np.float32)
        for g in range(1, NUM_KV):
            acc = acc + results[b * NUM_KV + g]["y"].astype(np.float32)
        out[b] = acc + bo[None, :]
    return out


def kernel(x, Wq, bq, Wk, bk, Wv, bv, Wo, bo):
    nc = build_nc()
    in_maps = make_in_maps(x, Wq, bq, Wk, bk, Wv, bv, Wo, bo)
    last = None
    for attempt in range(3):
        try:
            res = run_bass_kernel_spmd(nc, in_maps, list(range(8)))
            return gather(res.results, bo)
        except Exception as e:  # transient NRT_EXEC_UNIT_UNRECOVERABLE
            last = e
            import time as _t
            _t.sleep(10)
    raise last



# revision 3
# speedup vs baseline: 18915.5045x; 1.3290x over previous
"""GQA attention kernel for Trainium2, 8 NeuronCores.

Sharding: 2 batches x 4 kv-head groups = 8 cores. Each core computes, for its
batch b and kv group g (4 query heads, 1 kv head):
    Q = x_b @ Wq[:, g]   K = x_b @ Wk[:, g]   V = x_b @ Wv[:, g]
    A_h = softmax_causal(Q_h K^T / sqrt(128)) V        (h = 4 heads)
    Y_partial = concat_h(A_h) @ Wo[rows g]             [2048, 2048]
Host sums the 4 partials per batch and adds bo.

Optimizations over the naive schedule (all HW-slope verified):
  - upfront x-tile DMA prefetch (removes per-Tt PE stalls on x loads)
  - causal diagonal chunks shrink their query window (skips fully-masked
    columns); per-chunk 128-col band mask
  - softmax denominators accumulated on DVE in bf16 (kills 144 M=1 PE
    matmuls); single ones-matmul per (head, Tt)
  - PSUM evacuations on the Scalar engine (Identity/Copy) instead of DVE
  - per-head normalize chains deferred past the next head's first chunk
  - AV matmuls software-pipelined 2 chunks behind score matmuls
Compute dtype bf16, accumulation f32.
"""

import sys

sys.path.insert(0, "/opt/trn_rl_repo")

import numpy as np
import ml_dtypes

import concourse.bass as bass
import concourse.tile as tile
from concourse import mybir
from concourse.bass_utils import run_bass_kernel_spmd

BF = mybir.dt.bfloat16
F32 = mybir.dt.float32

D = 2048
T = 2048
B = 2
NUM_HEADS = 16
NUM_KV = 4
DH = 128
HPG = NUM_HEADS // NUM_KV
EG = HPG * DH
TS = 512
NT = T // TS
NJ = D // 128
SCALE = 1.0 / float(np.sqrt(DH))

CP_BASE = 2305
CP_COLS = CP_BASE + 128
AVLAG = 2

_NC_CACHE = {}


def _emit_iter(nc, tc, xT, Wq, Wk, Wv, Wo, y, ch):
    identity = ch["identity"]
    ones_s = ch["ones_s"]; ones_r = ch["ones_r"]; band = ch["band"]
    bq_sb = ch["bq_sb"]; bk_sb = ch["bk_sb"]; bv_sb = ch["bv_sb"]
    with (
        tc.tile_pool(name="persist", bufs=1) as persist,
        tc.tile_pool(name="wpool", bufs=1) as wpool,
        tc.tile_pool(name="xpool", bufs=64) as xpool,
        tc.tile_pool(name="expp", bufs=4) as expp,
        tc.tile_pool(name="attp", bufs=8) as attp,
        tc.tile_pool(name="ypool", bufs=4) as ypool,
        tc.tile_pool(name="small", bufs=8) as small,
    ):
        QT = [persist.tile([128, T], BF, tag=f"QT{h}", name=f"QT{h}") for h in range(HPG)]
        KT = persist.tile([128, T], BF, tag="KT")
        V = persist.tile([128, NJ, DH], BF, tag="V")
        Wq_sb = wpool.tile([128, NJ, EG], BF, tag="Wq")
        Wk_sb = wpool.tile([128, NJ, DH], BF, tag="Wk")
        Wv_sb = wpool.tile([128, NJ, DH], BF, tag="Wv")
        Wo_sb = wpool.tile([128, HPG, D], BF, tag="Wo")
        for h in range(HPG):
            nc.sync.dma_start(out=Wo_sb[:, h, :], in_=Wo[h * 128:(h + 1) * 128, :])

        # ---- phase A: projections QT/KT/V ------------------------------
        with (
            tc.tile_pool(name="psA", bufs=1, space="PSUM") as psA,
            tc.tile_pool(name="psAv", bufs=2, space="PSUM") as psAv,
        ):
            warm = psAv.tile([128, 128], BF, tag="v_ps")
            nc.tensor.transpose(warm, identity, identity)
            xa_pre = []
            for Tt in range(NT):
                tsl = slice(Tt * TS, (Tt + 1) * TS)
                row = []
                for j in range(NJ):
                    xt = xpool.tile([128, TS], BF, tag="xa")
                    nc.sync.dma_start(out=xt, in_=xT[j * 128:(j + 1) * 128, tsl])
                    row.append(xt)
                    if Tt == 0:
                        nc.sync.dma_start(out=Wq_sb[:, j, :], in_=Wq[j * 128:(j + 1) * 128, :])
                        nc.sync.dma_start(out=Wk_sb[:, j, :], in_=Wk[j * 128:(j + 1) * 128, :])
                        nc.sync.dma_start(out=Wv_sb[:, j, :], in_=Wv[j * 128:(j + 1) * 128, :])
                xa_pre.append(row)
            for Tt in range(NT):
                tsl = slice(Tt * TS, (Tt + 1) * TS)
                xa = xa_pre[Tt]
                for h in range(HPG):
                    qt_ps = psA.tile([128, TS], F32, tag=f"qt{h}")
                    for j in range(NJ):
                        nc.tensor.matmul(
                            qt_ps, Wq_sb[:, j, h * 128:(h + 1) * 128], xa[j],
                            start=(j == 0), stop=(j == NJ - 1),
                        )
                    nc.scalar.activation(
                        out=QT[h][:, tsl], in_=qt_ps,
                        func=mybir.ActivationFunctionType.Identity,
                        bias=bq_sb[:, h:h + 1])
                kt_ps = psA.tile([128, TS], F32, tag="kt")
                for j in range(NJ):
                    nc.tensor.matmul(kt_ps, Wk_sb[:, j, :], xa[j],
                                     start=(j == 0), stop=(j == NJ - 1))
                nc.scalar.activation(
                    out=KT[:, tsl], in_=kt_ps,
                    func=mybir.ActivationFunctionType.Identity, bias=bk_sb)
                vt_ps = psA.tile([128, TS], F32, tag="vt")
                for j in range(NJ):
                    nc.tensor.matmul(vt_ps, Wv_sb[:, j, :], xa[j],
                                     start=(j == 0), stop=(j == NJ - 1))
                vt_sb = small.tile([128, TS], BF, tag="vt_sb")
                nc.scalar.activation(
                    out=vt_sb, in_=vt_ps,
                    func=mybir.ActivationFunctionType.Identity, bias=bv_sb)
                for k in range(TS // 128):
                    v_ps = psAv.tile([128, 128], BF, tag="v_ps")
                    nc.tensor.transpose(v_ps, vt_sb[:, k * 128:(k + 1) * 128], identity)
                    nc.vector.tensor_copy(out=V[:, Tt * 4 + k, :], in_=v_ps)

        # ---- phase B/C: attention + output projection ------------------
        with (
            tc.tile_pool(name="psst", bufs=3, space="PSUM") as psst,
            tc.tile_pool(name="psat", bufs=2, space="PSUM") as psat,
            tc.tile_pool(name="psz", bufs=1, space="PSUM") as psz,
            tc.tile_pool(name="psy", bufs=2, space="PSUM") as psy,
        ):
            for Tt in range(NT):
                q0 = Tt * TS
                tsl = slice(q0, q0 + TS)
                att_sb = []
                pending = [None]

                def flush():
                    if pending[0] is not None:
                        pending[0]()
                        pending[0] = None

                for h in range(HPG):
                    njj = 4 * Tt + 4
                    at_ps = psat.tile([128, TS], F32, tag="at")
                    z_ps = psz.tile([1, TS], F32, tag="z")
                    nfull = njj - 4
                    gi = 0
                    exsum = expp.tile([128, TS], BF, tag="exs")
                    av_q = []

                    def emit_av(fn):
                        av_q.append(fn)
                        if len(av_q) > AVLAG:
                            av_q.pop(0)()

                    def drain_av():
                        while av_q:
                            av_q.pop(0)()

                    for j in range(nfull):
                        st = psst.tile([128, 512], F32, tag="st")
                        nc.tensor.matmul(
                            st, KT[:, j * 128:(j + 1) * 128],
                            QT[h][:, tsl], start=True, stop=True)
                        ex = expp.tile([128, 512], BF, tag="ex")
                        nc.scalar.activation(
                            out=ex, in_=st,
                            func=mybir.ActivationFunctionType.Exp, scale=SCALE)

                        def body(j=j, ex=ex, exsum=exsum, at_ps=at_ps):
                            if j == 0:
                                nc.vector.tensor_copy(out=exsum, in_=ex)
                            else:
                                nc.vector.tensor_add(exsum, exsum, ex)
                            nc.tensor.matmul(at_ps, V[:, j, :], ex,
                                             start=(j == 0), stop=False)
                        emit_av(body)
                        gi += 1
                        if gi == 2:
                            flush()
                    for k in range(4):
                        j = nfull + k
                        w = TS - 128 * k
                        st = psst.tile([128, 512], F32, tag="st")
                        ex = expp.tile([128, 512], BF, tag="ex")
                        nc.tensor.matmul(
                            st[:, 0:w], KT[:, j * 128:(j + 1) * 128],
                            QT[h][:, q0 + 128 * k:q0 + TS],
                            start=True, stop=True)
                        nc.scalar.activation(
                            out=ex[:, 0:w], in_=st[:, 0:w],
                            func=mybir.ActivationFunctionType.Exp, scale=SCALE)
                        nc.vector.tensor_mul(ex[:, 0:128], ex[:, 0:128], band)

                        def body(j=j, k=k, w=w, ex=ex, nfull=nfull, Tt=Tt,
                                 exsum=exsum, at_ps=at_ps):
                            first = (Tt == 0 and k == 0)
                            last = (k == 3)
                            if nfull == 0 and k == 0:
                                nc.vector.tensor_copy(out=exsum[:, 0:w], in_=ex[:, 0:w])
                            else:
                                nc.vector.tensor_add(
                                    exsum[:, 128 * k:TS], exsum[:, 128 * k:TS],
                                    ex[:, 0:w])
                            nc.tensor.matmul(
                                at_ps[:, 128 * k:TS], V[:, j, :], ex[:, 0:w],
                                start=first, stop=last)
                        emit_av(body)
                        gi += 1
                        if gi == 2:
                            flush()
                    drain_av()
                    nc.tensor.matmul(z_ps, ones_s, exsum, start=True, stop=True)

                    def chain(at_ps=at_ps, z_ps=z_ps):
                        zr = small.tile([1, TS], F32, tag="zr")
                        nc.vector.reciprocal(out=zr, in_=z_ps)
                        zrb = small.tile([1, TS], BF, tag="zrb")
                        nc.vector.tensor_copy(out=zrb, in_=zr)
                        zb_ps = psy.tile([128, TS], F32, tag="y")
                        nc.tensor.matmul(zb_ps, ones_r, zrb, start=True, stop=True)
                        zb_sb = small.tile([128, TS], BF, tag="zb_sb")
                        nc.vector.tensor_copy(out=zb_sb, in_=zb_ps)
                        at_sb = attp.tile([128, TS], BF, tag="at_sb")
                        nc.vector.tensor_mul(at_sb, at_ps, zb_sb)
                        att_sb.append(at_sb)
                    pending[0] = chain
                flush()
                for fs in range(4):
                    fsl = slice(fs * 512, (fs + 1) * 512)
                    for tt in range(4):
                        y_ps = psy.tile([128, 512], F32, tag="y")
                        for h in range(HPG):
                            nc.tensor.matmul(
                                y_ps,
                                att_sb[h][:, tt * 128:(tt + 1) * 128],
                                Wo_sb[:, h, fsl],
                                start=(h == 0), stop=(h == HPG - 1))
                        y_sb = ypool.tile([128, 512], F32, tag="y_sb")
                        nc.scalar.activation(
                            out=y_sb, in_=y_ps,
                            func=mybir.ActivationFunctionType.Copy)
                        nc.sync.dma_start(
                            out=y[Tt * TS + tt * 128: Tt * TS + (tt + 1) * 128, fsl],
                            in_=y_sb)


def build_nc(n_iters=1):
    key = ("nc", n_iters)
    if key in _NC_CACHE:
        return _NC_CACHE[key]
    nc = bass.Bass()
    xT = nc.dram_tensor("xT", [D, T], BF, kind="ExternalInput").ap()
    Wq = nc.dram_tensor("Wq", [D, EG], BF, kind="ExternalInput").ap()
    Wk = nc.dram_tensor("Wk", [D, DH], BF, kind="ExternalInput").ap()
    Wv = nc.dram_tensor("Wv", [D, DH], BF, kind="ExternalInput").ap()
    Wo = nc.dram_tensor("Wo", [EG, D], BF, kind="ExternalInput").ap()
    cpack_d = nc.dram_tensor("cpack", [128, CP_COLS], BF, kind="ExternalInput").ap()
    bpack_d = nc.dram_tensor("bpack", [128, 6], F32, kind="ExternalInput").ap()
    y = nc.dram_tensor("y", [T, D], F32, kind="ExternalOutput").ap()

    with tile.TileContext(nc) as tc:
        with tc.tile_pool(name="consts", bufs=1) as consts:
            cpack = consts.tile([128, CP_COLS], BF)
            nc.sync.dma_start(out=cpack, in_=cpack_d)
            bpack = consts.tile([128, 6], F32)
            nc.sync.dma_start(out=bpack, in_=bpack_d)
            ch = {
                "identity": cpack[:, 0:128],
                "ones_s": cpack[:, 2176:2177],
                "ones_r": cpack[0:1, 2177:2305],
                "band": cpack[:, CP_BASE:CP_BASE + 128],
                "bq_sb": bpack[:, 0:HPG],
                "bk_sb": bpack[:, HPG:HPG + 1],
                "bv_sb": bpack[:, HPG + 1:HPG + 2],
            }
            pt = consts.tile([128, 16], BF)
            nc.vector.tensor_copy(out=pt, in_=cpack[:, 0:16])
            ptf = consts.tile([128, 6], F32)
            nc.vector.tensor_copy(out=ptf, in_=bpack)

            for _ in range(n_iters):
                _emit_iter(nc, tc, xT, Wq, Wk, Wv, Wo, y, ch)
    from concourse.bacc import _bass_rust
    _bass_rust.move_matmul_waits_to_ldweights(nc.m)
    _bass_rust.generate_event_semaphores(nc)
    _NC_CACHE[key] = nc
    return nc


def _cpack():
    bf = ml_dtypes.bfloat16
    ident = np.eye(128, dtype=np.float32)
    tc_ = np.arange(512)[None, :]
    s = np.arange(128)[:, None]
    def mk(o0, o1):
        return np.concatenate(
            [(tc_ >= o0 * 128 + s), (tc_ >= o1 * 128 + s)], axis=1
        ).astype(np.float32)
    ones = np.ones((128, 129), np.float32)
    u = np.arange(128)[None, :]
    band = (u >= s).astype(np.float32)
    return np.concatenate([ident, mk(0, 1), mk(2, 3), ones, band],
                          axis=1).astype(bf)


def make_in_maps(x, Wq, bq, Wk, bk, Wv, bv, Wo, bo):
    bf = ml_dtypes.bfloat16
    cpack = _cpack()
    xTb = [np.ascontiguousarray(np.asarray(x[b]).T).astype(bf) for b in range(B)]
    Wq = np.asarray(Wq); Wk = np.asarray(Wk); Wv = np.asarray(Wv); Wo = np.asarray(Wo)
    bq = np.asarray(bq, dtype=np.float32); bk = np.asarray(bk, dtype=np.float32)
    bv = np.asarray(bv, dtype=np.float32)
    in_maps = []
    for c in range(8):
        b, g = divmod(c, NUM_KV)
        in_maps.append({
            "xT": xTb[b],
            "Wq": np.ascontiguousarray(Wq[:, g * EG:(g + 1) * EG]).astype(bf),
            "Wk": np.ascontiguousarray(Wk[:, g * DH:(g + 1) * DH]).astype(bf),
            "Wv": np.ascontiguousarray(Wv[:, g * DH:(g + 1) * DH]).astype(bf),
            "Wo": np.ascontiguousarray(Wo[g * EG:(g + 1) * EG, :]).astype(bf),
            "cpack": cpack,
            "bpack": np.concatenate(
                [bq[g * EG:(g + 1) * EG].reshape(4, DH).T,
                 bk[g * DH:(g + 1) * DH].reshape(DH, 1),
                 bv[g * DH:(g + 1) * DH].reshape(DH, 1)], axis=1
            ).astype(np.float32),
        })
    return in_maps


def gather(results, bo):
    bo = np.asarray(bo, dtype=np.float32)
    out = np.empty((B, T, D), dtype=np.float32)
    for b in range(B):
        acc = results[b * NUM_KV]["y"].astype(np.float32)
        for g in range(1, NUM_KV):
            acc = acc + results[b * NUM_KV + g]["y"].astype(np.float32)
        out[b] = acc + bo[None, :]
    return out


def kernel(x, Wq, bq, Wk, bk, Wv, bv, Wo, bo):
    nc = build_nc()
    in_maps = make_in_maps(x, Wq, bq, Wk, bk, Wv, bv, Wo, bo)
    last = None
    for attempt in range(3):
        try:
            res = run_bass_kernel_spmd(nc, in_maps, list(range(8)))
            return gather(res.results, bo)
        except Exception as e:  # transient NRT_EXEC_UNIT_UNRECOVERABLE
            last = e
            import time as _t
            _t.sleep(10)
    raise last
